# revision 7
# baseline (speedup 1.0000x reference)
"""EulerRotaryAttention Trainium2 kernel.

Sharding: 8 cores = 2 (batch) x 4 (head groups of 4 heads).  Each core
computes the qkv projection for its heads, rotary attention, and a partial
o-projection; the host sums partials over the 4 head groups per batch.

Device dataflow (zero on-device transposes):
  - x^T arrives pre-transposed from the host as (d, n).
  - Q^T, K^T computed directly in (feat, tok) layout with the projection
    weights as the stationary matmul operand.
  - RoPE rotation applied during PSUM eviction on VectorE.  Features are
    host-permuted (de-interleaved) so rotation pairs sit 32 partitions
    apart; cos/sin tables are host-precomputed (replicating the reference
    fp32 arithmetic exactly).
  - S^T in (k, q) layout (k on partitions, q free), causal tiles only.
  - exp on ScalarE (scores ~ N(0,1): no max subtraction needed); the
    diagonal 128x128 subtile is masked multiplicatively with a 0/1
    triangle; fully-masked column spans are memset to 0 on GpSimd.
  - PV: lhsT = [V | 1] so the PSUM accumulator yields both A^T (feats on
    partitions, q free) and the softmax denominators in one pass.
  - 1/denom via ScalarE exp(-log(d)); broadcast across 64 partitions on
    GpSimd; normalization fused into the A^T PSUM eviction on VectorE.
  - o-projection consumes A^T directly as lhsT; the partial (n, d) output
    is written per core and summed on the host.
"""

import math

import numpy as np

B, N, D, H = 2, 2048, 1024, 16
DH = D // H  # 64
HL = 4  # local heads per core
DL = HL * DH  # 256 local features
KC = D // 128  # 8 contraction chunks
NT = N // 128  # 16 token tiles
QC = N // 512  # 4 q-chunks
NCORES = 8

EULER_BASIS = (1.0, math.pi, math.e, math.pi * math.e, math.pi / math.e)

_PROG = None
LAST_RESULTS = None


def _build_program():
    import concourse.mybir as mybir
    import concourse.tile as tile
    from concourse import bacc

    f32 = mybir.dt.float32
    AF = mybir.ActivationFunctionType

    nc = bacc.Bacc("TRN2", target_bir_lowering=False, num_devices=NCORES)

    xT = nc.declare_dram_parameter("xT", [128, KC, N], f32, isOutput=False)
    wq = nc.declare_dram_parameter("wq", [128, KC, DL], f32, isOutput=False)
    wk = nc.declare_dram_parameter("wk", [128, KC, DL], f32, isOutput=False)
    wv = nc.declare_dram_parameter("wv", [128, KC, DL], f32, isOutput=False)
    wo = nc.declare_dram_parameter("wo", [128, 2, D], f32, isOutput=False)
    ctab = nc.declare_dram_parameter("ctab", [128, 2, N], f32, isOutput=False)
    stab = nc.declare_dram_parameter("stab", [128, 2, N], f32, isOutput=False)
    tri = nc.declare_dram_parameter("tri", [128, 128], f32, isOutput=False)
    o_out = nc.declare_dram_parameter("o_out", [NT, 128, D], f32, isOutput=True)

    with tile.TileContext(nc) as tc:
        with tc.tile_pool(name="persist", bufs=1) as persist:
            # rotated Q^T / K^T: (256 feats, N) as 2 x (128, N)
            qt_rot = [persist.tile([128, N], f32, tag=f"qt{m}", name=f"qt{m}") for m in range(2)]
            kt_rot = [persist.tile([128, N], f32, tag=f"kt{m}", name=f"kt{m}") for m in range(2)]
            # V with appended ones column, per head: (128, NT, 65)
            vones = [
                persist.tile([128, NT, DH + 1], f32, tag=f"v{h}", name=f"v{h}") for h in range(HL)
            ]
            for h in range(HL):
                nc.vector.memset(vones[h][:, :, DH : DH + 1], 1.0)
            # A^T head pairs: (128, N)
            at2 = [persist.tile([128, N], f32, tag=f"at{m}", name=f"at{m}") for m in range(2)]

            # ================= phase 1: projections =================
            with (
                tc.tile_pool(name="p1c", bufs=1) as p1c,
                tc.tile_pool(name="rot_tmp", bufs=2) as rot_tmp,
                tc.tile_pool(name="psum_qkt", bufs=2, space="PSUM") as psum_qkt,
                tc.tile_pool(name="psum_v", bufs=2, space="PSUM") as psum_v,
            ):
                wq_sb = p1c.tile([128, KC, DL], f32, tag="wq")
                wk_sb = p1c.tile([128, KC, DL], f32, tag="wk")
                wv_sb = p1c.tile([128, KC, DL], f32, tag="wv")
                ctab_sb = p1c.tile([128, 2, N], f32, tag="ctab")
                stab_sb = p1c.tile([128, 2, N], f32, tag="stab")
                nc.sync.dma_start(out=wq_sb[:], in_=wq[:])
                nc.sync.dma_start(out=wk_sb[:], in_=wk[:])
                nc.sync.dma_start(out=wv_sb[:], in_=wv[:])
                nc.sync.dma_start(out=ctab_sb[:], in_=ctab[:])
                nc.sync.dma_start(out=stab_sb[:], in_=stab[:])

                xT_sb = p1c.tile([128, KC, N], f32, tag="xT")
                for kc in range(KC):
                    nc.sync.dma_start(out=xT_sb[:, kc, :], in_=xT[:, kc, :])

                # Q^T / K^T: lhsT = w[kc, feats], rhs = xT[kc, toks]
                for w_sb, rot in ((wq_sb, qt_rot), (wk_sb, kt_rot)):
                    for mt in range(2):
                        for nh in range(2):  # halves of N (1024 = 2 psum banks)
                            psum = psum_qkt.tile([128, 1024], f32, tag="qkt")
                            for kc in range(KC):
                                for nq in range(2):
                                    nc.tensor.matmul(
                                        psum[:, nq * 512 : (nq + 1) * 512],
                                        w_sb[:, kc, mt * 128 : (mt + 1) * 128],
                                        xT_sb[
                                            :,
                                            kc,
                                            nh * 1024
                                            + nq * 512 : nh * 1024
                                            + (nq + 1) * 512,
                                        ],
                                        start=(kc == 0),
                                        stop=(kc == KC - 1),
                                    )
                            # rotation eviction:
                            #   rot = psum * ctab + swap32(psum) * stab
                            for nq in range(2):
                                fsl = slice(nq * 512, (nq + 1) * 512)
                                nsl = slice(
                                    nh * 1024 + nq * 512, nh * 1024 + (nq + 1) * 512
                                )
                                nc.vector.tensor_mul(
                                    rot[mt][:, nsl], psum[:, fsl], ctab_sb[:, mt, nsl]
                                )
                                tmp = rot_tmp.tile([128, 512], f32, tag="rt")
                                for g in range(4):
                                    s = g ^ 1
                                    nc.vector.tensor_mul(
                                        tmp[g * 32 : (g + 1) * 32, :],
                                        psum[s * 32 : (s + 1) * 32, fsl],
                                        stab_sb[g * 32 : (g + 1) * 32, mt, nsl],
                                    )
                                nc.vector.tensor_add(
                                    rot[mt][:, nsl], rot[mt][:, nsl], tmp[:]
                                )

                # V: lhsT = xT[kc, toks], rhs = wv[kc, feats]
                for tt in range(NT):
                    vpsum = psum_v.tile([128, DL], f32, tag="v")
                    for kc in range(KC):
                        nc.tensor.matmul(
                            vpsum[:],
                            xT_sb[:, kc, tt * 128 : (tt + 1) * 128],
                            wv_sb[:, kc, :],
                            start=(kc == 0),
                            stop=(kc == KC - 1),
                        )
                    for h in range(HL):
                        nc.scalar.copy(
                            out=vones[h][:, tt, 0:DH],
                            in_=vpsum[:, h * DH : (h + 1) * DH],
                        )

            # ============ phase 2: attention + o-projection ============
            with (
                tc.tile_pool(name="p2c", bufs=1) as p2c,
                tc.tile_pool(name="exps_pool", bufs=20) as exps_pool,
                tc.tile_pool(name="norm_pool", bufs=4) as norm_pool,
                tc.tile_pool(name="bcast_pool", bufs=6) as bcast_pool,
                tc.tile_pool(name="ostage_pool", bufs=3) as ostage_pool,
                tc.tile_pool(name="psum_s", bufs=3, space="PSUM") as psum_s,
                tc.tile_pool(name="psum_pv", bufs=2, space="PSUM") as psum_pv,
                tc.tile_pool(name="psum_o", bufs=1, space="PSUM") as psum_o,
            ):
                wo_sb = p2c.tile([128, 2, D], f32, tag="wo")
                tri_sb = p2c.tile([128, 128], f32, tag="tri")
                nc.sync.dma_start(out=wo_sb[:], in_=wo[:])
                nc.sync.dma_start(out=tri_sb[:], in_=tri[:])

                for qc in range(QC):
                    qsl = slice(qc * 512, (qc + 1) * 512)
                    for h in range(HL):
                        mt, roff = h // 2, (h % 2) * 64
                        nkt = 4 * qc + 4
                        # all S^T tiles + exp for this (h, qc) first ...
                        exps_tiles = []
                        for kt in range(nkt):
                            spsum = psum_s.tile([128, 512], f32, tag="s", name="spsum")
                            nc.tensor.matmul(
                                spsum[:],
                                kt_rot[mt][roff : roff + 64, kt * 128 : (kt + 1) * 128],
                                qt_rot[mt][roff : roff + 64, qsl],
                                start=True,
                                stop=True,
                            )
                            exps = exps_pool.tile([128, 512], f32, tag="e", name="exps")
                            exps_tiles.append(exps)
                            j = kt - 4 * qc
                            if j < 0:
                                nc.scalar.activation(exps[:], spsum[:], AF.Exp)
                            else:
                                jo = j * 128
                                nc.scalar.activation(
                                    exps[:, jo:512], spsum[:, jo:512], AF.Exp
                                )
                                nc.vector.tensor_mul(
                                    exps[:, jo : jo + 128],
                                    exps[:, jo : jo + 128],
                                    tri_sb[:],
                                )
                                if j > 0:
                                    nc.gpsimd.memset(exps[:, 0:jo], 0.0)
                        # ... then the PV accumulation as one uninterrupted group
                        pv = psum_pv.tile([DH + 1, 512], f32, tag="pv", name="pv")
                        for kt in range(nkt):
                            nc.tensor.matmul(
                                pv[:],
                                vones[h][:, kt, :],
                                exps_tiles[kt][:],
                                start=(kt == 0),
                                stop=(kt == nkt - 1),
                            )
                        # 1/denom = exp(-log(denom)) on ScalarE, off the PSUM row
                        rcp = norm_pool.tile([1, 512], f32, tag="rcp", name="rcp")
                        nc.scalar.activation(rcp[:], pv[DH : DH + 1, :], AF.Ln)
                        nc.scalar.activation(rcp[:], rcp[:], AF.Exp, scale=-1.0)
                        bc = bcast_pool.tile([DH, 512], f32, tag="bc", name="bc")
                        nc.gpsimd.partition_broadcast(bc[:], rcp[:])
                        nc.vector.tensor_mul(
                            at2[mt][roff : roff + DH, qsl], pv[0:DH, :], bc[:]
                        )

                # o-projection: out[tok, :] = sum_feat A^T[feat, tok] * wo[feat, :]
                for tt in range(NT):
                    opsum = psum_o.tile([128, D], f32, tag="o")
                    for hp in range(2):
                        for nb in range(2):
                            nc.tensor.matmul(
                                opsum[:, nb * 512 : (nb + 1) * 512],
                                at2[hp][:, tt * 128 : (tt + 1) * 128],
                                wo_sb[:, hp, nb * 512 : (nb + 1) * 512],
                                start=(hp == 0),
                                stop=(hp == 1),
                            )
                    ost = ostage_pool.tile([128, D], f32, tag="ost")
                    nc.scalar.copy(out=ost[:, 0:512], in_=opsum[:, 0:512])
                    nc.vector.tensor_copy(out=ost[:, 512:D], in_=opsum[:, 512:D])
                    nc.sync.dma_start(out=o_out[tt], in_=ost[:])

    nc.compile()
    return nc


def get_program():
    global _PROG
    if _PROG is None:
        _PROG = _build_program()
    return _PROG


def _host_tables(bit_logits):
    """Replicate the reference fp32 cos/sin computation exactly (jax on CPU)."""
    import jax

    with jax.default_device(jax.devices("cpu")[0]):
        import jax.numpy as jnp

        basis = jnp.asarray(EULER_BASIS, dtype=jnp.float32)
        freqs = jax.nn.sigmoid(jnp.asarray(bit_logits, dtype=jnp.float32)) @ basis
        inv_freq = 2.0 ** (-(jnp.arange(0, DH, 2, dtype=jnp.float32) / DH))
        pos = jnp.arange(N, dtype=jnp.float32)
        theta = pos[None, :, None] * freqs[:, None, None] * inv_freq[None, None, :]
        cos = np.asarray(jnp.cos(theta))  # (H, N, 32)
        sin = np.asarray(jnp.sin(theta))
    return cos, sin


def _chunk_rows(a, p=128):
    """(R, C) -> (p, R//p, C); row r = kc*p + pp lands at [pp, kc]."""
    r, c = a.shape
    return np.ascontiguousarray(a.reshape(r // p, p, c).transpose(1, 0, 2))


def prepare_inputs(x, w_qkv, w_o, bit_logits):
    x = np.asarray(x, dtype=np.float32)
    w_qkv = np.asarray(w_qkv, dtype=np.float32)
    w_o = np.asarray(w_o, dtype=np.float32)
    cos, sin = _host_tables(np.asarray(bit_logits, dtype=np.float32))

    # de-interleave permutation within a head: evens then odds
    perm = np.concatenate([np.arange(0, DH, 2), np.arange(1, DH, 2)])

    wq_full = w_qkv.reshape(D, 3, H, DH)[:, 0]  # (D, H, DH)
    wk_full = w_qkv.reshape(D, 3, H, DH)[:, 1]
    wv_full = w_qkv.reshape(D, 3, H, DH)[:, 2]
    scale = 1.0 / math.sqrt(DH)

    # tri[krow, qcol] = 1 if qcol >= krow else 0  (allowed = q >= k)
    tri = np.triu(np.ones((128, 128), dtype=np.float32))

    xT_by_batch = [
        _chunk_rows(np.ascontiguousarray(x[b].T)) for b in range(B)
    ]  # (128, KC, N)

    per_group = []
    for g in range(4):
        heads = range(4 * g, 4 * g + 4)
        wq_g = np.concatenate(
            [wq_full[:, h][:, perm] * scale for h in heads], axis=1
        )  # (D, 256)
        wk_g = np.concatenate([wk_full[:, h][:, perm] for h in heads], axis=1)
        wv_g = np.concatenate([wv_full[:, h] for h in heads], axis=1)
        wo_g = np.concatenate(
            [w_o.reshape(H, DH, D)[h] for h in heads], axis=0
        )  # (256, D)

        # rotation tables, layout (256 feats, N) -> (128, 2, N)
        ct = np.empty((DL, N), dtype=np.float32)
        st = np.empty((DL, N), dtype=np.float32)
        for hl, h in enumerate(heads):
            c = cos[h].T  # (32, N)
            s = sin[h].T
            ct[hl * DH : hl * DH + 32] = c
            ct[hl * DH + 32 : hl * DH + 64] = c
            st[hl * DH : hl * DH + 32] = -s
            st[hl * DH + 32 : hl * DH + 64] = s
        per_group.append(
            dict(
                wq=_chunk_rows(wq_g),
                wk=_chunk_rows(wk_g),
                wv=_chunk_rows(wv_g),
                wo=_chunk_rows(wo_g),
                ctab=_chunk_rows(ct),
                stab=_chunk_rows(st),
                tri=tri,
            )
        )

    in_maps = []
    for c in range(NCORES):
        b, g = c // 4, c % 4
        m = dict(per_group[g])
        m["xT"] = xT_by_batch[b]
        in_maps.append(m)
    return in_maps


def kernel(x, w_qkv, w_o, bit_logits, n_heads):
    global LAST_RESULTS
    from concourse.bass_utils import run_bass_kernel_spmd

    assert int(n_heads) == H
    nc = get_program()
    in_maps = prepare_inputs(x, w_qkv, w_o, bit_logits)
    res = run_bass_kernel_spmd(nc, in_maps, list(range(NCORES)))
    LAST_RESULTS = res
    out = np.zeros((B, N, D), dtype=np.float32)
    for c in range(NCORES):
        b = c // 4
        out[b] += res.results[c]["o_out"].reshape(N, D)
    return out


# revision 10
# speedup vs baseline: 1.7058x; 1.7058x over previous
"""EulerRotaryAttention Trainium2 kernel (bf16 matmul pipeline).

Sharding: 8 cores = 2 (batch) x 4 (head groups of 4 heads).  Each core
computes the qkv projection for its heads, rotary attention, and a partial
o-projection; the host sums partials over the 4 head groups per batch.

Device dataflow (zero on-device transposes):
  - x^T arrives pre-transposed from the host as (d, n), bf16.
  - Q^T, K^T computed directly in (feat, tok) layout with the projection
    weights as the stationary matmul operand; fp32 PSUM accumulation.
  - RoPE rotation applied during PSUM eviction.  Features are
    host-permuted (de-interleaved) so rotation pairs sit 32 partitions
    apart; cos/sin tables host-precomputed (replicating the reference
    fp32 arithmetic).  The swap-half products run on GpSimd, the rest on
    VectorE in bf16 2x mode.
  - S^T in (k, q) layout (k on partitions, q free), causal tiles only,
    bf16 operands, 1024-wide q chunks.
  - exp on ScalarE (scores ~ N(0,1): no max subtraction needed); the
    diagonal 128x128 subtile is masked multiplicatively with a 0/1
    triangle; fully-masked column spans are memset to 0 on GpSimd.
  - PV: lhsT = [V | 1] (bf16) so the fp32 PSUM accumulator yields both
    A^T (feats on partitions, q free) and the softmax denominators.
  - 1/denom via ScalarE exp(-log(d)); broadcast across 64 partitions on
    GpSimd; normalization fused into the A^T PSUM eviction on VectorE.
  - o-projection consumes A^T directly as lhsT; the partial (n, d) fp32
    output is written per core and summed on the host.
"""

import math

import numpy as np

B, N, D, H = 2, 2048, 1024, 16
DH = D // H  # 64
HL = 4  # local heads per core
DL = HL * DH  # 256 local features
KC = D // 128  # 8 contraction chunks
NT = N // 128  # 16 token tiles
NCH = N // 1024  # 2 wide column chunks
NCORES = 8

EULER_BASIS = (1.0, math.pi, math.e, math.pi * math.e, math.pi / math.e)

_PROG = None
LAST_RESULTS = None


def _build_program():
    import concourse.mybir as mybir
    import concourse.tile as tile
    from concourse import bacc

    f32 = mybir.dt.float32
    bf = mybir.dt.bfloat16
    AF = mybir.ActivationFunctionType

    nc = bacc.Bacc("TRN2", target_bir_lowering=False, num_devices=NCORES)

    xT = nc.declare_dram_parameter("xT", [128, KC, N], bf, isOutput=False)
    wq = nc.declare_dram_parameter("wq", [128, KC, DL], bf, isOutput=False)
    wk = nc.declare_dram_parameter("wk", [128, KC, DL], bf, isOutput=False)
    wv = nc.declare_dram_parameter("wv", [128, KC, DL], bf, isOutput=False)
    wo = nc.declare_dram_parameter("wo", [128, 2, D], bf, isOutput=False)
    ctab = nc.declare_dram_parameter("ctab", [128, 2, N], bf, isOutput=False)
    stab = nc.declare_dram_parameter("stab", [128, 2, N], bf, isOutput=False)
    tri = nc.declare_dram_parameter("tri", [128, 128], bf, isOutput=False)
    o_out = nc.declare_dram_parameter("o_out", [NT, 128, D], f32, isOutput=True)

    with tile.TileContext(nc) as tc:
        with tc.tile_pool(name="persist", bufs=1) as persist:
            # rotated Q^T / K^T: (256 feats, N) as 2 x (128, N), bf16
            qt_rot = [
                persist.tile([128, N], bf, tag=f"qt{m}", name=f"qt{m}")
                for m in range(2)
            ]
            kt_rot = [
                persist.tile([128, N], bf, tag=f"kt{m}", name=f"kt{m}")
                for m in range(2)
            ]
            # V with appended ones column, per head: (128, NT, 65) bf16
            vones = [
                persist.tile([128, NT, DH + 1], bf, tag=f"v{h}", name=f"v{h}")
                for h in range(HL)
            ]
            for h in range(HL):
                nc.vector.memset(vones[h][:, :, DH : DH + 1], 1.0)
            # A^T head pairs: (128, N) bf16
            at2 = [
                persist.tile([128, N], bf, tag=f"at{m}", name=f"at{m}")
                for m in range(2)
            ]

            # ================= phase 1: projections =================
            with (
                tc.tile_pool(name="p1c", bufs=1) as p1c,
                tc.tile_pool(name="raw_pool", bufs=3) as raw_pool,
                tc.tile_pool(name="rot_tmp", bufs=3) as rot_tmp,
                tc.tile_pool(name="psum_qkt", bufs=2, space="PSUM") as psum_qkt,
                tc.tile_pool(name="psum_v", bufs=2, space="PSUM") as psum_v,
            ):
                wq_sb = p1c.tile([128, KC, DL], bf, tag="wq")
                wk_sb = p1c.tile([128, KC, DL], bf, tag="wk")
                wv_sb = p1c.tile([128, KC, DL], bf, tag="wv")
                ctab_sb = p1c.tile([128, 2, N], bf, tag="ctab")
                stab_sb = p1c.tile([128, 2, N], bf, tag="stab")
                nc.sync.dma_start(out=wq_sb[:], in_=wq[:])
                nc.sync.dma_start(out=wk_sb[:], in_=wk[:])
                nc.sync.dma_start(out=wv_sb[:], in_=wv[:])
                nc.sync.dma_start(out=ctab_sb[:], in_=ctab[:])
                nc.sync.dma_start(out=stab_sb[:], in_=stab[:])

                xT_sb = p1c.tile([128, KC, N], bf, tag="xT")
                for kc in range(KC):
                    nc.sync.dma_start(out=xT_sb[:, kc, :], in_=xT[:, kc, :])

                # Q^T / K^T: lhsT = w[kc, feats], rhs = xT[kc, toks]
                for w_sb, rot in ((wq_sb, qt_rot), (wk_sb, kt_rot)):
                    for mt in range(2):
                        for nh in range(NCH):  # 1024-wide tok chunks
                            nsl = slice(nh * 1024, (nh + 1) * 1024)
                            psum = psum_qkt.tile([128, 1024], f32, tag="qkt")
                            for kc in range(KC):
                                for nq in range(2):
                                    nc.tensor.matmul(
                                        psum[:, nq * 512 : (nq + 1) * 512],
                                        w_sb[:, kc, mt * 128 : (mt + 1) * 128],
                                        xT_sb[
                                            :,
                                            kc,
                                            nh * 1024
                                            + nq * 512 : nh * 1024
                                            + (nq + 1) * 512,
                                        ],
                                        start=(kc == 0),
                                        stop=(kc == KC - 1),
                                    )
                            # rotation eviction:
                            #   rot = raw * ctab + swap32(raw) * stab
                            raw = raw_pool.tile([128, 1024], bf, tag="raw")
                            nc.vector.tensor_copy(raw[:], psum[:])
                            nc.vector.tensor_mul(
                                rot[mt][:, nsl], raw[:], ctab_sb[:, mt, nsl]
                            )
                            # swap 32-row halves via single-input copies
                            # (both-SBUF tensor_tensor needs equal base
                            # partitions), then one aligned multiply
                            raws = rot_tmp.tile([128, 1024], bf, tag="rs", name="raws")
                            for g in range(4):
                                s = g ^ 1
                                nc.gpsimd.tensor_copy(
                                    raws[g * 32 : (g + 1) * 32, :],
                                    raw[s * 32 : (s + 1) * 32, :],
                                )
                            tmp = rot_tmp.tile([128, 1024], bf, tag="rt", name="tmp")
                            nc.gpsimd.tensor_mul(
                                tmp[:], raws[:], stab_sb[:, mt, nsl]
                            )
                            nc.vector.tensor_add(
                                rot[mt][:, nsl], rot[mt][:, nsl], tmp[:]
                            )

                # V: lhsT = xT[kc, toks], rhs = wv[kc, feats]
                for tt in range(NT):
                    vpsum = psum_v.tile([128, DL], f32, tag="v")
                    for kc in range(KC):
                        nc.tensor.matmul(
                            vpsum[:],
                            xT_sb[:, kc, tt * 128 : (tt + 1) * 128],
                            wv_sb[:, kc, :],
                            start=(kc == 0),
                            stop=(kc == KC - 1),
                        )
                    for h in range(HL):
                        nc.scalar.copy(
                            out=vones[h][:, tt, 0:DH],
                            in_=vpsum[:, h * DH : (h + 1) * DH],
                        )

            # ============ phase 2a: attention ============
            with (
                tc.tile_pool(name="p2c", bufs=1) as p2c,
                tc.tile_pool(name="exps_pool", bufs=18) as exps_pool,
                tc.tile_pool(name="norm_pool", bufs=4) as norm_pool,
                tc.tile_pool(name="bcast_pool", bufs=4) as bcast_pool,
                tc.tile_pool(name="psum_s", bufs=2, space="PSUM") as psum_s,
                tc.tile_pool(name="psum_pv", bufs=2, space="PSUM") as psum_pv,
            ):
                tri_sb = p2c.tile([128, 128], bf, tag="tri")
                nc.sync.dma_start(out=tri_sb[:], in_=tri[:])

                for qch in range(NCH):
                    qsl = slice(qch * 1024, (qch + 1) * 1024)
                    for h in range(HL):
                        mt, roff = h // 2, (h % 2) * 64
                        nkt = 8 * qch + 8
                        # all S^T tiles + exp for this (h, qch) first ...
                        exps_tiles = []
                        for kt in range(nkt):
                            spsum = psum_s.tile([128, 1024], f32, tag="s", name="spsum")
                            for nq in range(2):
                                nc.tensor.matmul(
                                    spsum[:, nq * 512 : (nq + 1) * 512],
                                    kt_rot[mt][
                                        roff : roff + 64, kt * 128 : (kt + 1) * 128
                                    ],
                                    qt_rot[mt][
                                        roff : roff + 64,
                                        qch * 1024
                                        + nq * 512 : qch * 1024
                                        + (nq + 1) * 512,
                                    ],
                                    start=True,
                                    stop=True,
                                )
                            exps = exps_pool.tile([128, 1024], bf, tag="e", name="exps")
                            exps_tiles.append(exps)
                            j = kt - 8 * qch
                            if j < 0:
                                nc.scalar.activation(exps[:], spsum[:], AF.Exp)
                            else:
                                jo = j * 128
                                nc.scalar.activation(
                                    exps[:, jo:1024], spsum[:, jo:1024], AF.Exp
                                )
                                nc.vector.tensor_mul(
                                    exps[:, jo : jo + 128],
                                    exps[:, jo : jo + 128],
                                    tri_sb[:],
                                )
                                if j > 0:
                                    nc.gpsimd.memset(exps[:, 0:jo], 0.0)
                        # ... then the PV accumulation as one uninterrupted group
                        pv = psum_pv.tile([DH + 1, 1024], f32, tag="pv", name="pv")
                        for kt in range(nkt):
                            for nq in range(2):
                                nc.tensor.matmul(
                                    pv[:, nq * 512 : (nq + 1) * 512],
                                    vones[h][:, kt, :],
                                    exps_tiles[kt][:, nq * 512 : (nq + 1) * 512],
                                    start=(kt == 0),
                                    stop=(kt == nkt - 1),
                                )
                        # 1/denom = exp(-log(denom)) on ScalarE, off the PSUM row
                        rcp = norm_pool.tile([1, 1024], f32, tag="rcp", name="rcp")
                        nc.scalar.activation(rcp[:], pv[DH : DH + 1, :], AF.Ln)
                        nc.scalar.activation(rcp[:], rcp[:], AF.Exp, scale=-1.0)
                        bc = bcast_pool.tile([DH, 1024], f32, tag="bc", name="bc")
                        nc.gpsimd.partition_broadcast(bc[:], rcp[:])
                        nc.vector.tensor_mul(
                            at2[mt][roff : roff + DH, qsl], pv[0:DH, :], bc[:]
                        )

            # ============ phase 2b: o-projection ============
            with (
                tc.tile_pool(name="p2d", bufs=1) as p2d,
                tc.tile_pool(name="ostage_pool", bufs=3) as ostage_pool,
                tc.tile_pool(name="psum_o", bufs=2, space="PSUM") as psum_o,
            ):
                wo_sb = p2d.tile([128, 2, D], bf, tag="wo")
                nc.sync.dma_start(out=wo_sb[:], in_=wo[:])

                for tt in range(NT):
                    opsum = psum_o.tile([128, D], f32, tag="o")
                    for hp in range(2):
                        for nb in range(2):
                            nc.tensor.matmul(
                                opsum[:, nb * 512 : (nb + 1) * 512],
                                at2[hp][:, tt * 128 : (tt + 1) * 128],
                                wo_sb[:, hp, nb * 512 : (nb + 1) * 512],
                                start=(hp == 0),
                                stop=(hp == 1),
                            )
                    ost = ostage_pool.tile([128, D], f32, tag="ost")
                    nc.scalar.copy(out=ost[:, 0:512], in_=opsum[:, 0:512])
                    nc.vector.tensor_copy(out=ost[:, 512:D], in_=opsum[:, 512:D])
                    nc.sync.dma_start(out=o_out[tt], in_=ost[:])

    nc.compile()
    return nc


def get_program():
    global _PROG
    if _PROG is None:
        _PROG = _build_program()
    return _PROG


def _host_tables(bit_logits):
    """Replicate the reference fp32 cos/sin computation exactly (jax on CPU)."""
    import jax

    with jax.default_device(jax.devices("cpu")[0]):
        import jax.numpy as jnp

        basis = jnp.asarray(EULER_BASIS, dtype=jnp.float32)
        freqs = jax.nn.sigmoid(jnp.asarray(bit_logits, dtype=jnp.float32)) @ basis
        inv_freq = 2.0 ** (-(jnp.arange(0, DH, 2, dtype=jnp.float32) / DH))
        pos = jnp.arange(N, dtype=jnp.float32)
        theta = pos[None, :, None] * freqs[:, None, None] * inv_freq[None, None, :]
        cos = np.asarray(jnp.cos(theta))  # (H, N, 32)
        sin = np.asarray(jnp.sin(theta))
    return cos, sin


def _chunk_rows(a, p=128):
    """(R, C) -> (p, R//p, C); row r = kc*p + pp lands at [pp, kc]."""
    r, c = a.shape
    return np.ascontiguousarray(a.reshape(r // p, p, c).transpose(1, 0, 2))


def prepare_inputs(x, w_qkv, w_o, bit_logits):
    import ml_dtypes

    bf = ml_dtypes.bfloat16

    x = np.asarray(x, dtype=np.float32)
    w_qkv = np.asarray(w_qkv, dtype=np.float32)
    w_o = np.asarray(w_o, dtype=np.float32)
    cos, sin = _host_tables(np.asarray(bit_logits, dtype=np.float32))

    # de-interleave permutation within a head: evens then odds
    perm = np.concatenate([np.arange(0, DH, 2), np.arange(1, DH, 2)])

    wq_full = w_qkv.reshape(D, 3, H, DH)[:, 0]  # (D, H, DH)
    wk_full = w_qkv.reshape(D, 3, H, DH)[:, 1]
    wv_full = w_qkv.reshape(D, 3, H, DH)[:, 2]
    scale = 1.0 / math.sqrt(DH)

    # tri[krow, qcol] = 1 if qcol >= krow else 0  (allowed = q >= k)
    tri = np.triu(np.ones((128, 128), dtype=np.float32))

    xT_by_batch = [
        _chunk_rows(np.ascontiguousarray(x[b].T)) for b in range(B)
    ]  # (128, KC, N)

    per_group = []
    for g in range(4):
        heads = range(4 * g, 4 * g + 4)
        wq_g = np.concatenate(
            [wq_full[:, h][:, perm] * scale for h in heads], axis=1
        )  # (D, 256)
        wk_g = np.concatenate([wk_full[:, h][:, perm] for h in heads], axis=1)
        wv_g = np.concatenate([wv_full[:, h] for h in heads], axis=1)
        wo_g = np.concatenate(
            [w_o.reshape(H, DH, D)[h] for h in heads], axis=0
        )  # (256, D)

        # rotation tables, layout (256 feats, N) -> (128, 2, N)
        ct = np.empty((DL, N), dtype=np.float32)
        st = np.empty((DL, N), dtype=np.float32)
        for hl, h in enumerate(heads):
            c = cos[h].T  # (32, N)
            s = sin[h].T
            ct[hl * DH : hl * DH + 32] = c
            ct[hl * DH + 32 : hl * DH + 64] = c
            st[hl * DH : hl * DH + 32] = -s
            st[hl * DH + 32 : hl * DH + 64] = s
        per_group.append(
            dict(
                wq=_chunk_rows(wq_g).astype(bf),
                wk=_chunk_rows(wk_g).astype(bf),
                wv=_chunk_rows(wv_g).astype(bf),
                wo=_chunk_rows(wo_g).astype(bf),
                ctab=_chunk_rows(ct).astype(bf),
                stab=_chunk_rows(st).astype(bf),
                tri=tri.astype(bf),
            )
        )

    in_maps = []
    for c in range(NCORES):
        b, g = c // 4, c % 4
        m = dict(per_group[g])
        m["xT"] = xT_by_batch[b].astype(bf)
        in_maps.append(m)
    return in_maps


def kernel(x, w_qkv, w_o, bit_logits, n_heads):
    global LAST_RESULTS
    from concourse.bass_utils import run_bass_kernel_spmd

    assert int(n_heads) == H
    nc = get_program()
    in_maps = prepare_inputs(x, w_qkv, w_o, bit_logits)
    res = run_bass_kernel_spmd(nc, in_maps, list(range(NCORES)))
    LAST_RESULTS = res
    out = np.zeros((B, N, D), dtype=np.float32)
    for c in range(NCORES):
        b = c // 4
        out[b] += res.results[c]["o_out"].reshape(N, D)
    return out


# revision 16
# speedup vs baseline: 3.0371x; 1.7804x over previous
"""EulerRotaryAttention Trainium2 kernel (bf16 matmul pipeline).

Sharding: 8 cores = 2 (batch) x 4 (head groups of 4 heads).  Each core
computes the qkv projection for its heads, rotary attention, and a partial
o-projection; the host sums partials over the 4 head groups per batch.

Device dataflow (zero on-device transposes):
  - x^T arrives pre-transposed from the host as (d, n), bf16.
  - Q^T, K^T computed directly in (feat, tok) layout with the projection
    weights as the stationary matmul operand; fp32 PSUM accumulation.
  - RoPE rotation applied during PSUM eviction.  Features are
    host-permuted (de-interleaved) so rotation pairs sit 32 partitions
    apart; cos/sin tables host-precomputed (replicating the reference
    fp32 arithmetic).  PSUM->bf16 cast on ScalarE, swap-half copies and
    multiply/add on VectorE in bf16 fast modes.
  - S^T in (k, q) layout (k on partitions, q free), causal tiles only;
    matmul streams are clipped to the causal column range per PSUM bank.
  - exp on ScalarE (scores ~ N(0,1): no max subtraction needed) into one
    (128, kt, 1024) bf16 tile per (head, q-chunk); the 8 diagonal 128x128
    subtiles are masked with a single strided tensor_tensor against a
    replicated 0/1 triangle.
  - PV: lhsT = [V | 1] (bf16) so the fp32 PSUM accumulator yields both
    A^T (feats on partitions, q free) and the softmax denominators.
  - denominators: batched VectorE reciprocal (no Ln -> only the Exp ACT
    table is ever loaded); GpSimd broadcast across partitions pairs two
    heads into one (128, 1024) scale tile; one in-place multiply
    normalizes each A^T head pair.
  - o-projection consumes A^T directly as lhsT; the partial (n, d) fp32
    output is written per core and summed on the host.
"""

import math

import numpy as np

B, N, D, H = 2, 2048, 1024, 16
DH = D // H  # 64
HL = 4  # local heads per core
DL = HL * DH  # 256 local features
KC = D // 128  # 8 contraction chunks
NT = N // 128  # 16 token tiles
NCH = N // 1024  # 2 wide column chunks
NCORES = 8

EULER_BASIS = (1.0, math.pi, math.e, math.pi * math.e, math.pi / math.e)

_PROG = None
LAST_RESULTS = None


def _build_program():
    import concourse.bass as bass
    import concourse.mybir as mybir
    import concourse.tile as tile
    from concourse import bacc

    f32 = mybir.dt.float32
    bf = mybir.dt.bfloat16
    AF = mybir.ActivationFunctionType

    nc = bacc.Bacc("TRN2", target_bir_lowering=False, num_devices=NCORES)

    xT = nc.declare_dram_parameter("xT", [128, KC, N], bf, isOutput=False)
    wq = nc.declare_dram_parameter("wq", [128, KC, DL], bf, isOutput=False)
    wk = nc.declare_dram_parameter("wk", [128, KC, DL], bf, isOutput=False)
    wv = nc.declare_dram_parameter("wv", [128, KC, DL], bf, isOutput=False)
    wo = nc.declare_dram_parameter("wo", [128, 2, D], bf, isOutput=False)
    ctab = nc.declare_dram_parameter("ctab", [128, 2, N], bf, isOutput=False)
    stab = nc.declare_dram_parameter("stab", [128, 2, N], bf, isOutput=False)
    tri8 = nc.declare_dram_parameter("tri8", [128, 8, 128], bf, isOutput=False)
    o_out = nc.declare_dram_parameter("o_out", [NT, 128, D], f32, isOutput=True)

    with tile.TileContext(nc) as tc:
        with tc.tile_pool(name="persist", bufs=1) as persist:
            # rotated Q^T / K^T: (256 feats, N) as 2 x (128, N), bf16
            qt_rot = [
                persist.tile([128, N], bf, tag=f"qt{m}", name=f"qt{m}")
                for m in range(2)
            ]
            kt_rot = [
                persist.tile([128, N], bf, tag=f"kt{m}", name=f"kt{m}")
                for m in range(2)
            ]
            # V for all heads with appended ones column: (128, NT, HL, 65)
            vones = persist.tile([128, NT, HL, DH + 1], bf, tag="vones", name="vones")
            nc.vector.memset(vones[:, :, :, DH : DH + 1], 1.0)
            # A^T head pairs: (128, N) bf16
            at2 = [
                persist.tile([128, N], bf, tag=f"at{m}", name=f"at{m}")
                for m in range(2)
            ]

            # ================= phase 1: projections =================
            with (
                tc.tile_pool(name="p1c", bufs=1) as p1c,
                tc.tile_pool(name="rot_tmp", bufs=3) as rot_tmp,
                tc.tile_pool(name="psum_qkt", bufs=2, space="PSUM") as psum_qkt,
                tc.tile_pool(name="psum_v", bufs=2, space="PSUM") as psum_v,
            ):
                wq_sb = p1c.tile([128, KC, DL], bf, tag="wq")
                wk_sb = p1c.tile([128, KC, DL], bf, tag="wk")
                wv_sb = p1c.tile([128, KC, DL], bf, tag="wv")
                ctab_sb = p1c.tile([128, 2, N], bf, tag="ctab")
                stab_sb = p1c.tile([128, 2, N], bf, tag="stab")
                nc.sync.dma_start(out=wq_sb[:], in_=wq[:])
                nc.sync.dma_start(out=wk_sb[:], in_=wk[:])
                nc.sync.dma_start(out=wv_sb[:], in_=wv[:])
                nc.sync.dma_start(out=ctab_sb[:], in_=ctab[:])
                nc.sync.dma_start(out=stab_sb[:], in_=stab[:])

                xT_sb = p1c.tile([128, KC, N], bf, tag="xT")
                for kc in range(KC):
                    nc.sync.dma_start(out=xT_sb[:, kc, :], in_=xT[:, kc, :])

                # Q^T / K^T: lhsT = w[kc, feats], rhs = xT[kc, toks]
                for w_sb, rot in ((wq_sb, qt_rot), (wk_sb, kt_rot)):
                    for mt in range(2):
                        for nh in range(NCH):  # 1024-wide tok chunks
                            nsl = slice(nh * 1024, (nh + 1) * 1024)
                            psum = psum_qkt.tile([128, 1024], f32, tag="qkt")
                            for kc in range(KC):
                                for nq in range(2):
                                    nc.tensor.matmul(
                                        psum[:, nq * 512 : (nq + 1) * 512],
                                        w_sb[:, kc, mt * 128 : (mt + 1) * 128],
                                        xT_sb[
                                            :,
                                            kc,
                                            nh * 1024
                                            + nq * 512 : nh * 1024
                                            + (nq + 1) * 512,
                                        ],
                                        start=(kc == 0),
                                        stop=(kc == KC - 1),
                                    )
                            # rotation eviction:
                            #   rot = raw * ctab + swap32(raw) * stab
                            raw = rot_tmp.tile([128, 1024], bf, tag="raw", name="raw")
                            nc.scalar.copy(out=raw[:], in_=psum[:])
                            nc.vector.tensor_mul(
                                rot[mt][:, nsl], raw[:], ctab_sb[:, mt, nsl]
                            )
                            raws = rot_tmp.tile([128, 1024], bf, tag="rs", name="raws")
                            for g in range(4):
                                s = g ^ 1
                                nc.vector.tensor_copy(
                                    raws[g * 32 : (g + 1) * 32, :],
                                    raw[s * 32 : (s + 1) * 32, :],
                                )
                            tmp = rot_tmp.tile([128, 1024], bf, tag="rt", name="tmp")
                            nc.vector.tensor_mul(tmp[:], raws[:], stab_sb[:, mt, nsl])
                            nc.vector.tensor_add(
                                rot[mt][:, nsl], rot[mt][:, nsl], tmp[:]
                            )

                # V: lhsT = xT[kc, toks], rhs = wv[kc, feats]
                for tt in range(NT):
                    vpsum = psum_v.tile([128, DL], f32, tag="v")
                    for kc in range(KC):
                        nc.tensor.matmul(
                            vpsum[:],
                            xT_sb[:, kc, tt * 128 : (tt + 1) * 128],
                            wv_sb[:, kc, :],
                            start=(kc == 0),
                            stop=(kc == KC - 1),
                        )
                    # single strided eviction for all 4 heads of this tile
                    nc.scalar.copy(
                        out=vones[:, tt, :, 0:DH],
                        in_=vpsum[:].rearrange("p (h d) -> p h d", h=HL),
                    )

            # ============ phase 2a: attention ============
            with (
                tc.tile_pool(name="p2c", bufs=1) as p2c,
                tc.tile_pool(name="exps_pool", bufs=2) as exps_pool,
                tc.tile_pool(name="norm_pool", bufs=2) as norm_pool,
                tc.tile_pool(name="bcast_pool", bufs=2) as bcast_pool,
                tc.tile_pool(name="dscr_pool", bufs=4, space="DRAM") as dscr_pool,
                tc.tile_pool(name="psum_s", bufs=2, space="PSUM") as psum_s,
                tc.tile_pool(name="psum_pv", bufs=2, space="PSUM") as psum_pv,
            ):
                tri8_sb = p2c.tile([128, 8, 128], bf, tag="tri8")
                nc.sync.dma_start(out=tri8_sb[:], in_=tri8[:])

                for qch in range(NCH):
                    qsl = slice(qch * 1024, (qch + 1) * 1024)
                    nkt = 8 * qch + 8
                    # denominator rows live at partitions 0/32/64/96 (the
                    # only legal engine start partitions); unused rows are
                    # memset to 1.0 so the batched reciprocal stays finite
                    dnm4 = norm_pool.tile([97, 1024], f32, tag="dnm", name="dnm4")
                    rcp4 = norm_pool.tile([97, 1024], f32, tag="rcp", name="rcp4")
                    nc.gpsimd.memset(dnm4[:], 1.0)
                    for h in range(HL):
                        mt, roff = h // 2, (h % 2) * 64
                        exps = exps_pool.tile([128, NT, 1024], bf, tag="e", name="exps")
                        for kt in range(nkt):
                            j = kt - 8 * qch
                            jo = max(j, 0) * 128
                            spsum = psum_s.tile([128, 1024], f32, tag="s", name="spsum")
                            for nq in range(2):
                                lo = max(jo, nq * 512)
                                hi = (nq + 1) * 512
                                if lo >= hi:
                                    continue
                                nc.tensor.matmul(
                                    spsum[:, lo:hi],
                                    kt_rot[mt][
                                        roff : roff + 64, kt * 128 : (kt + 1) * 128
                                    ],
                                    qt_rot[mt][
                                        roff : roff + 64,
                                        qch * 1024 + lo : qch * 1024 + hi,
                                    ],
                                    start=True,
                                    stop=True,
                                )
                            nc.scalar.activation(
                                exps[:, kt, jo:1024], spsum[:, jo:1024], AF.Exp
                            )
                        # mask all 8 diagonal 128x128 subtiles in one op:
                        # element (p, j, c) -> exps[p, 8*qch + j, j*128 + c]
                        sub = exps[:, 8 * qch, :]
                        diag = bass.AP(
                            tensor=sub.tensor,
                            offset=sub.offset,
                            ap=[list(sub.ap[0]), [1152, 8], [1, 128]],
                        )
                        nc.vector.tensor_mul(diag, diag, tri8_sb[:])
                        # PV accumulation as one uninterrupted group
                        pv = psum_pv.tile([DH + 1, 1024], f32, tag="pv", name="pv")
                        # last kt contributing to each 512-col bank
                        last_kt = (8 * qch + 3, 8 * qch + 7)
                        for kt in range(nkt):
                            j = kt - 8 * qch
                            jo = max(j, 0) * 128
                            for nq in range(2):
                                lo = max(jo, nq * 512)
                                hi = (nq + 1) * 512
                                if lo >= hi:
                                    continue
                                nc.tensor.matmul(
                                    pv[:, lo:hi],
                                    vones[:, kt, h, :],
                                    exps[:, kt, lo:hi],
                                    start=(kt == 0),
                                    stop=(kt == last_kt[nq]),
                                )
                        # stash denominator, evict unnormalized A^T
                        nc.scalar.copy(
                            out=dnm4[32 * h : 32 * h + 1, :],
                            in_=pv[DH : DH + 1, :],
                        )
                        nc.vector.tensor_copy(
                            out=at2[mt][roff : roff + DH, qsl], in_=pv[0:DH, :]
                        )
                    # batched reciprocal of the 4 denominators
                    nc.vector.reciprocal(rcp4[:], dnm4[:])
                    for mt in range(2):
                        bc = bcast_pool.tile([128, 1024], f32, tag="bc", name="bc")
                        # broadcast each head's reciprocal row across 64
                        # partitions: bounce through DRAM, then a step-0
                        # partition DMA (legal for DRAM sources only; POOL's
                        # partition_broadcast ignores non-zero base
                        # partitions on hardware)
                        for half in range(2):
                            row = rcp4[64 * mt + 32 * half : 64 * mt + 32 * half + 1, :]
                            rdram = dscr_pool.tile([1, 1024], f32, tag="rd", name="rd")
                            nc.sync.dma_start(out=rdram[:], in_=row)
                            rd = rdram[:]
                            nc.sync.dma_start(
                                out=bc[64 * half : 64 * half + 64, :],
                                in_=bass.AP(
                                    tensor=rd.tensor,
                                    offset=rd.offset,
                                    ap=[[0, 64], [1, 1024]],
                                ),
                            )
                        nc.vector.tensor_mul(at2[mt][:, qsl], at2[mt][:, qsl], bc[:])

            # ============ phase 2b: o-projection ============
            with (
                tc.tile_pool(name="p2d", bufs=1) as p2d,
                tc.tile_pool(name="ostage_pool", bufs=3) as ostage_pool,
                tc.tile_pool(name="psum_o", bufs=2, space="PSUM") as psum_o,
            ):
                wo_sb = p2d.tile([128, 2, D], bf, tag="wo")
                nc.sync.dma_start(out=wo_sb[:], in_=wo[:])

                for tt in range(NT):
                    opsum = psum_o.tile([128, D], f32, tag="o")
                    for hp in range(2):
                        for nb in range(2):
                            nc.tensor.matmul(
                                opsum[:, nb * 512 : (nb + 1) * 512],
                                at2[hp][:, tt * 128 : (tt + 1) * 128],
                                wo_sb[:, hp, nb * 512 : (nb + 1) * 512],
                                start=(hp == 0),
                                stop=(hp == 1),
                            )
                    ost = ostage_pool.tile([128, D], f32, tag="ost")
                    nc.scalar.copy(out=ost[:, 0:512], in_=opsum[:, 0:512])
                    nc.vector.tensor_copy(out=ost[:, 512:D], in_=opsum[:, 512:D])
                    nc.sync.dma_start(out=o_out[tt], in_=ost[:])

    nc.compile()
    return nc


def get_program():
    global _PROG
    if _PROG is None:
        _PROG = _build_program()
    return _PROG


def _host_tables(bit_logits):
    """Replicate the reference fp32 cos/sin computation exactly (jax on CPU)."""
    import jax

    with jax.default_device(jax.devices("cpu")[0]):
        import jax.numpy as jnp

        basis = jnp.asarray(EULER_BASIS, dtype=jnp.float32)
        freqs = jax.nn.sigmoid(jnp.asarray(bit_logits, dtype=jnp.float32)) @ basis
        inv_freq = 2.0 ** (-(jnp.arange(0, DH, 2, dtype=jnp.float32) / DH))
        pos = jnp.arange(N, dtype=jnp.float32)
        theta = pos[None, :, None] * freqs[:, None, None] * inv_freq[None, None, :]
        cos = np.asarray(jnp.cos(theta))  # (H, N, 32)
        sin = np.asarray(jnp.sin(theta))
    return cos, sin


def _chunk_rows(a, p=128):
    """(R, C) -> (p, R//p, C); row r = kc*p + pp lands at [pp, kc]."""
    r, c = a.shape
    return np.ascontiguousarray(a.reshape(r // p, p, c).transpose(1, 0, 2))


def prepare_inputs(x, w_qkv, w_o, bit_logits):
    import ml_dtypes

    bf = ml_dtypes.bfloat16

    x = np.asarray(x, dtype=np.float32)
    w_qkv = np.asarray(w_qkv, dtype=np.float32)
    w_o = np.asarray(w_o, dtype=np.float32)
    cos, sin = _host_tables(np.asarray(bit_logits, dtype=np.float32))

    # de-interleave permutation within a head: evens then odds
    perm = np.concatenate([np.arange(0, DH, 2), np.arange(1, DH, 2)])

    wq_full = w_qkv.reshape(D, 3, H, DH)[:, 0]  # (D, H, DH)
    wk_full = w_qkv.reshape(D, 3, H, DH)[:, 1]
    wv_full = w_qkv.reshape(D, 3, H, DH)[:, 2]
    scale = 1.0 / math.sqrt(DH)

    # tri[krow, qcol] = 1 if qcol >= krow else 0, replicated 8x for the
    # strided diagonal mask
    tri = np.triu(np.ones((128, 128), dtype=np.float32))
    tri8 = np.broadcast_to(tri[:, None, :], (128, 8, 128)).copy()

    xT_by_batch = [
        _chunk_rows(np.ascontiguousarray(x[b].T)) for b in range(B)
    ]  # (128, KC, N)

    per_group = []
    for g in range(4):
        heads = range(4 * g, 4 * g + 4)
        wq_g = np.concatenate(
            [wq_full[:, h][:, perm] * scale for h in heads], axis=1
        )  # (D, 256)
        wk_g = np.concatenate([wk_full[:, h][:, perm] for h in heads], axis=1)
        wv_g = np.concatenate([wv_full[:, h] for h in heads], axis=1)
        wo_g = np.concatenate(
            [w_o.reshape(H, DH, D)[h] for h in heads], axis=0
        )  # (256, D)

        # rotation tables, layout (256 feats, N) -> (128, 2, N)
        ct = np.empty((DL, N), dtype=np.float32)
        st = np.empty((DL, N), dtype=np.float32)
        for hl, h in enumerate(heads):
            c = cos[h].T  # (32, N)
            s = sin[h].T
            ct[hl * DH : hl * DH + 32] = c
            ct[hl * DH + 32 : hl * DH + 64] = c
            st[hl * DH : hl * DH + 32] = -s
            st[hl * DH + 32 : hl * DH + 64] = s
        per_group.append(
            dict(
                wq=_chunk_rows(wq_g).astype(bf),
                wk=_chunk_rows(wk_g).astype(bf),
                wv=_chunk_rows(wv_g).astype(bf),
                wo=_chunk_rows(wo_g).astype(bf),
                ctab=_chunk_rows(ct).astype(bf),
                stab=_chunk_rows(st).astype(bf),
                tri8=tri8.astype(bf),
            )
        )

    in_maps = []
    for c in range(NCORES):
        b, g = c // 4, c % 4
        m = dict(per_group[g])
        m["xT"] = xT_by_batch[b].astype(bf)
        in_maps.append(m)
    return in_maps


def kernel(x, w_qkv, w_o, bit_logits, n_heads):
    global LAST_RESULTS
    from concourse.bass_utils import run_bass_kernel_spmd

    assert int(n_heads) == H
    nc = get_program()
    in_maps = prepare_inputs(x, w_qkv, w_o, bit_logits)
    res = run_bass_kernel_spmd(nc, in_maps, list(range(NCORES)))
    LAST_RESULTS = res
    out = np.zeros((B, N, D), dtype=np.float32)
    for c in range(NCORES):
        b = c // 4
        out[b] += res.results[c]["o_out"].reshape(N, D)
    return out
